# revision 9
# baseline (speedup 1.0000x reference)
"""Trainium2 Bass kernel for nn_MixtureOfExpertsLayer (moe_routing).

Sharding: token-data-parallel across 8 NeuronCores (1024 tokens/core),
weights replicated.  All 4 experts are computed densely per core and the
top-2 softmax gating is applied as a per-token scalar during the final
down-projection PSUM eviction (scalar_tensor_tensor accumulate), so no
gather/scatter is needed.

Layouts: activations are feature-major [128p, feature_chunk, token] so
mid-chain matmuls use the weight chunk as the stationary operand; the
final matmul of each expert flips roles (activation chunk stationary) to
produce token-major [token 128p, H] partials accumulated into `acc`.

Dtypes: float32r (tf32-like, ~1e-3 precision, full PE rate at N=512) for
expert compute; plain fp32 for the router so the top-2 selection matches
the fp32 reference.  LN stats via ones-matmul; [1,N] -> [128,N]
broadcasts via K=1 ones matmul into PSUM.
"""
import numpy as np

import concourse.bass as bass
import concourse.mybir as mybir
import concourse.tile as tile
from concourse import bacc
from concourse.alu_op_type import AluOpType
from concourse.bass_utils import run_bass_kernel_spmd

F32 = mybir.dt.float32
F32R = mybir.dt.float32r
ACT = mybir.ActivationFunctionType
AX = mybir.AxisListType
OP = AluOpType

N_CORES = 8
B, S, H, I, E = 4, 2048, 1024, 4096, 4
TOK = (B * S) // N_CORES  # tokens per core
P = 128

WEIGHT_NAMES = [
    "router_w", "router_b", "load_balancer",
    "sw_w1", "sw_w3", "sw_w2",
    "me_eq_w", "me_eq_b", "me_wv", "me_bv", "me_wo", "me_bo",
    "me_c1w", "me_c1b", "me_c2w", "me_c2b",
    "ce_syn_w", "ce_syn_b", "ce_wv", "ce_bv", "ce_wo", "ce_bo",
    "ce_n1g", "ce_n1b", "ce_f1w", "ce_f1b", "ce_f2w", "ce_f2b",
    "ce_n2g", "ce_n2b", "ce_gen_w", "ce_gen_b",
    "ml_w1", "ml_b1", "ml_w2", "ml_b2",
]


def build_moe(h=H, i_dim=I, tok=TOK):
    KC = h // P              # contraction chunks for H
    KC2 = (2 * h) // P
    TT = tok // P            # token tiles
    NTH = tok // 512         # 512-token slabs
    IB = i_dim // 512        # 512-row blocks of I
    HB = h // 512

    nc = bacc.Bacc("TRN2", target_bir_lowering=False, debug=False)

    def din(name, shape):
        return nc.dram_tensor(name, shape, F32, kind="ExternalInput")

    xt = din("xt", [h, tok])
    dt = {}
    shapes = {
        "router_w": [h, E], "router_b": [E], "load_balancer": [E],
        "sw_w1": [h, i_dim], "sw_w3": [h, i_dim], "sw_w2": [i_dim, h],
        "me_eq_w": [h, h], "me_eq_b": [h], "me_wv": [h, h], "me_bv": [h],
        "me_wo": [h, h], "me_bo": [h], "me_c1w": [h, 2 * h], "me_c1b": [2 * h],
        "me_c2w": [2 * h, h], "me_c2b": [h],
        "ce_syn_w": [h, h], "ce_syn_b": [h], "ce_wv": [h, h], "ce_bv": [h],
        "ce_wo": [h, h], "ce_bo": [h], "ce_n1g": [h], "ce_n1b": [h],
        "ce_f1w": [h, 2 * h], "ce_f1b": [2 * h], "ce_f2w": [2 * h, h],
        "ce_f2b": [h], "ce_n2g": [h], "ce_n2b": [h],
        "ce_gen_w": [h, h], "ce_gen_b": [h],
        "ml_w1": [h, i_dim], "ml_b1": [i_dim], "ml_w2": [i_dim, h],
        "ml_b2": [h],
    }
    for n in WEIGHT_NAMES:
        dt[n] = din(n, shapes[n])
    out = nc.dram_tensor("out", [tok, h], F32, kind="ExternalOutput")

    def wap(w):  # [rows, cols] -> [p, row_chunk, cols]
        return w.ap().rearrange("(kc p) m -> p kc m", p=P)

    def bap(b):  # [dim] -> [p, chunk]
        return b.ap().rearrange("(mc p) -> p mc", p=P)

    cast_rr = [0]

    with tile.TileContext(nc) as tc:
        with (
            tc.tile_pool(name="const", bufs=1) as cpool,
            tc.tile_pool(name="persist", bufs=1) as ppool,
            tc.tile_pool(name="big", bufs=3) as bigp,
            tc.tile_pool(name="blk", bufs=2) as blkp,
            tc.tile_pool(name="wst", bufs=2) as wpool,
            tc.tile_pool(name="wsr", bufs=2) as wrpool,
            tc.tile_pool(name="lns", bufs=1) as lnsp,
            tc.tile_pool(name="tmp", bufs=2) as tmpp,
            tc.tile_pool(name="ps", bufs=4, space=bass.MemorySpace.PSUM) as psp,
            tc.tile_pool(name="pss", bufs=2, space=bass.MemorySpace.PSUM) as pssp,
            tc.tile_pool(name="psb", bufs=2, space=bass.MemorySpace.PSUM) as psbp,
        ):
            def cast(dst, src):
                """fp32 -> fp32r cast, alternating DVE / ACT."""
                cast_rr[0] ^= 1
                if cast_rr[0]:
                    nc.vector.tensor_copy(dst, src)
                else:
                    nc.scalar.activation(dst, src, ACT.Copy)

            # ---- constants ---------------------------------------------
            ones_cf = cpool.tile([P, 1], F32, tag="ones_cf")
            nc.vector.memset(ones_cf[:], 1.0)
            ones_c = cpool.tile([P, 1], F32R, tag="ones_c")
            nc.vector.tensor_copy(ones_c[:], ones_cf[:])
            ones_rf = cpool.tile([1, P], F32, tag="ones_rf")
            nc.vector.memset(ones_rf[:], 1.0)
            ones_r = cpool.tile([1, P], F32R, tag="ones_r")
            nc.vector.tensor_copy(ones_r[:], ones_rf[:])

            def const_bias(name, mc):
                t = cpool.tile([P, mc], F32, tag=name + "_cb")
                nc.sync.dma_start(t[:], bap(dt[name]))
                return t

            def const_row_r(name, n):
                tf = tmpp.tile([1, n], F32, tag="row_stage")
                nc.sync.dma_start(tf[:], dt[name].ap().unsqueeze(0))
                tr = cpool.tile([1, n], F32R, tag=name + "_rr")
                nc.vector.tensor_copy(tr[:], tf[:])
                return tr

            eq_b_t = const_bias("me_eq_b", KC)
            bv_t = const_bias("me_bv", KC)
            bo_t = const_bias("me_bo", KC)
            c1b_t = const_bias("me_c1b", KC2)
            syn_b_t = const_bias("ce_syn_b", KC)
            cbv_t = const_bias("ce_bv", KC)
            cbo_t = const_bias("ce_bo", KC)
            f1b_t = const_bias("ce_f1b", KC2)
            f2b_t = const_bias("ce_f2b", KC)
            ml_b1_t = const_bias("ml_b1", i_dim // P)
            n1g_t = const_bias("ce_n1g", KC)
            n1b_t = const_bias("ce_n1b", KC)
            n2g_t = const_bias("ce_n2g", KC)
            n2b_t = const_bias("ce_n2b", KC)
            c2b_row = const_row_r("me_c2b", h)
            gen_b_row = const_row_r("ce_gen_b", h)
            ml_b2_row = const_row_r("ml_b2", h)

            rb_f = cpool.tile([1, E], F32, tag="rb_f")
            nc.sync.dma_start(rb_f[:], dt["router_b"].ap().unsqueeze(0))
            lb_f = cpool.tile([1, E], F32, tag="lb_f")
            nc.sync.dma_start(lb_f[:], dt["load_balancer"].ap().unsqueeze(0))
            rblb = cpool.tile([1, E], F32, tag="rblb")
            nc.vector.tensor_tensor(rblb[:], rb_f[:], lb_f[:], OP.add)
            rw_sb = cpool.tile([P, KC, E], F32, tag="rw_sb")
            nc.sync.dma_start(rw_sb[:], wap(dt["router_w"]))

            # ---- persistent state --------------------------------------
            xr = ppool.tile([P, KC, tok], F32R, tag="xr")
            acc = ppool.tile([P, TT, h], F32, tag="acc")
            wgate = ppool.tile([P, TT, E], F32, tag="wgate")

            # ---- router + gating + x cast per 512-token slab -----------
            for sh in range(NTH):
                xf = bigp.tile([P, KC, 512], F32, tag="big")
                nc.sync.dma_start(xf[:], wap(xt)[:, :, sh * 512:(sh + 1) * 512])
                for tl in range(4):
                    t = sh * 4 + tl
                    lps = psp.tile([P, E], F32, tag="mm")
                    for kc in range(KC):
                        nc.tensor.matmul(lps[:], xf[:, kc, tl * P:(tl + 1) * P],
                                         rw_sb[:, kc, :],
                                         start=(kc == 0), stop=False)
                    nc.tensor.matmul(lps[:], ones_rf[:], rblb[:],
                                     start=False, stop=True)
                    m1 = tmpp.tile([P, 1], F32, tag="g1")
                    nc.vector.tensor_reduce(m1[:], lps[:], AX.X, OP.max)
                    ind1 = tmpp.tile([P, E], F32, tag="g2")
                    nc.vector.tensor_scalar(ind1[:], lps[:], m1[:], -1e30,
                                            OP.is_ge, OP.mult)
                    lm = tmpp.tile([P, E], F32, tag="g3")
                    nc.vector.tensor_tensor(lm[:], lps[:], ind1[:], OP.add)
                    m2 = tmpp.tile([P, 1], F32, tag="g4")
                    nc.vector.tensor_reduce(m2[:], lm[:], AX.X, OP.max)
                    nm1 = tmpp.tile([P, 1], F32, tag="g5")
                    nc.vector.tensor_scalar(nm1[:], m1[:], -1.0, None, OP.mult)
                    d = tmpp.tile([P, E], F32, tag="g6")
                    nc.vector.tensor_scalar(d[:], lps[:], nm1[:], None, OP.add)
                    ed = tmpp.tile([P, E], F32, tag="g7")
                    nc.scalar.activation(ed[:], d[:], ACT.Exp)
                    em = tmpp.tile([P, 1], F32, tag="g8")
                    nc.scalar.activation(em[:], m2[:], ACT.Exp, bias=nm1[:])
                    z = tmpp.tile([P, 1], F32, tag="g9")
                    nc.vector.tensor_scalar(z[:], em[:], 1.0, None, OP.add)
                    rz = tmpp.tile([P, 1], F32, tag="g10")
                    nc.vector.reciprocal(rz[:], z[:])
                    ind2 = tmpp.tile([P, E], F32, tag="g11")
                    nc.vector.tensor_scalar(ind2[:], lps[:], m2[:], None,
                                            OP.is_ge)
                    nc.vector.scalar_tensor_tensor(wgate[:, t, :], ed[:], rz[:],
                                                   ind2[:], OP.mult, OP.mult)
                cast(xr[:, :, sh * 512:(sh + 1) * 512], xf[:])

            # ---- helpers -----------------------------------------------
            def fm(dst, w_name, mc_out, src, src_off, act, bias_t, bias_col0=0,
                   w_col0=0):
                """dst[:, mc, :512] = act(W[:, cols].T @ src + b).

                Streams W in 256-column blocks; contraction over KC chunks
                of 128; 512 tokens starting at src_off."""
                w_all = wap(dt[w_name])
                for m0 in range(0, mc_out, 2):
                    wf = wpool.tile([P, KC, 256], F32, tag="w")
                    nc.sync.dma_start(
                        wf[:],
                        w_all[:, :, w_col0 + m0 * P:w_col0 + (m0 + 2) * P])
                    wr = wrpool.tile([P, KC, 256], F32R, tag="wr")
                    cast(wr[:], wf[:])
                    for ml in range(2):
                        mc = m0 + ml
                        ps = psp.tile([P, 512], F32, tag="mm")
                        for kc in range(KC):
                            nc.tensor.matmul(
                                ps[:], wr[:, kc, ml * P:(ml + 1) * P],
                                src[:, kc, src_off:src_off + 512],
                                start=(kc == 0), stop=(kc == KC - 1))
                        if bias_t is None:
                            nc.scalar.activation(dst[:, mc, :], ps[:], act)
                        else:
                            b_sl = bias_t[:, bias_col0 + mc:bias_col0 + mc + 1]
                            f = (ACT.Identity if act == ACT.Copy else act)
                            nc.scalar.activation(dst[:, mc, :], ps[:], f,
                                                 bias=b_sl)

            def tm(w_name, rb0, kcb, src, gate_col, tok_off, init,
                   bias_row=None):
                """acc[:, tt, :] (+)= wgate[:,:,gate_col] * (src.T @ W_rows
                [+ bias]).  src is [P, kcb, 512] feature-major; W rows
                rb0*128 .. (rb0+kcb)*128 stream in [P, 4, 512] blocks."""
                w_all = wap(dt[w_name])
                nkb = (kcb + 3) // 4
                for hh in range(HB):
                    wrs = []
                    for kb in range(nkb):
                        kw = min(4, kcb - kb * 4)
                        wf = wpool.tile([P, 4, 512], F32, tag="w")
                        nc.sync.dma_start(
                            wf[:, :kw, :],
                            w_all[:, rb0 + kb * 4:rb0 + kb * 4 + kw,
                                  hh * 512:(hh + 1) * 512])
                        wr = wrpool.tile([P, 4, 512], F32R, tag="wr")
                        cast(wr[:, :kw, :], wf[:, :kw, :])
                        wrs.append((wr, kw))
                    for tl in range(4):
                        tt = (tok_off // P) + tl
                        ps = psp.tile([P, 512], F32, tag="mm")
                        for kb, (wr, kw) in enumerate(wrs):
                            for kc in range(kw):
                                last = (kb == nkb - 1 and kc == kw - 1)
                                nc.tensor.matmul(
                                    ps[:],
                                    src[:, kb * 4 + kc, tl * P:(tl + 1) * P],
                                    wr[:, kc, :],
                                    start=(kb == 0 and kc == 0),
                                    stop=(last and bias_row is None))
                        if bias_row is not None:
                            nc.tensor.matmul(
                                ps[:], ones_r[:],
                                bias_row[0:1, hh * 512:(hh + 1) * 512],
                                start=False, stop=True)
                        a_sl = acc[:, tt, hh * 512:(hh + 1) * 512]
                        g_sl = wgate[:, tt, gate_col:gate_col + 1]
                        if init:
                            nc.vector.tensor_scalar(a_sl, ps[:], g_sl, None,
                                                    OP.mult)
                        else:
                            nc.vector.scalar_tensor_tensor(
                                a_sl, ps[:], g_sl, a_sl, OP.mult, OP.add)

            def layer_norm(dst, src, g_t, b_t):
                """dst = LN(src)*g + b over the feature dim (cross-chunk)."""
                ssum = pssp.tile([1, 512], F32, tag="st")
                for kc in range(KC):
                    nc.tensor.matmul(ssum[:], ones_c[:], src[:, kc, :],
                                     start=(kc == 0), stop=(kc == KC - 1))
                ssq = pssp.tile([1, 512], F32, tag="st")
                for half in range(KC // 4):
                    sq = blkp.tile([P, 4, 512], F32R, tag="blk")
                    nc.vector.tensor_tensor(
                        sq[:], src[:, half * 4:half * 4 + 4, :],
                        src[:, half * 4:half * 4 + 4, :], OP.mult)
                    for kc in range(4):
                        nc.tensor.matmul(ssq[:], ones_c[:], sq[:, kc, :],
                                         start=(half == 0 and kc == 0),
                                         stop=(half == KC // 4 - 1 and kc == 3))
                mu = lnsp.tile([1, 512], F32R, tag="ln1")
                nc.vector.tensor_scalar(mu[:], ssum[:], 1.0 / h, None, OP.mult)
                msq = lnsp.tile([1, 512], F32, tag="ln2")
                nc.vector.tensor_scalar(msq[:], ssq[:], 1.0 / h, None, OP.mult)
                mu2 = lnsp.tile([1, 512], F32, tag="ln3")
                nc.vector.tensor_tensor(mu2[:], mu[:], mu[:], OP.mult)
                var = lnsp.tile([1, 512], F32, tag="ln4")
                nc.vector.scalar_tensor_tensor(var[:], msq[:], 1e-5, mu2[:],
                                               OP.add, OP.subtract)
                sdev = lnsp.tile([1, 512], F32, tag="ln5a")
                nc.scalar.activation(sdev[:], var[:], ACT.Sqrt)
                rstd_f = lnsp.tile([1, 512], F32, tag="ln5f")
                nc.vector.reciprocal(rstd_f[:], sdev[:])
                rstd = lnsp.tile([1, 512], F32R, tag="ln5")
                nc.vector.tensor_copy(rstd[:], rstd_f[:])
                mub = psbp.tile([P, 512], F32, tag="bc")
                nc.tensor.matmul(mub[:], ones_r[:], mu[:], start=True,
                                 stop=True)
                rsb = psbp.tile([P, 512], F32, tag="bc")
                nc.tensor.matmul(rsb[:], ones_r[:], rstd[:], start=True,
                                 stop=True)
                for kc in range(KC):
                    t1 = tmpp.tile([P, 512], F32, tag="lnt")
                    nc.vector.tensor_tensor(t1[:], src[:, kc, :], mub[:],
                                            OP.subtract)
                    nc.vector.tensor_tensor(t1[:], t1[:], rsb[:], OP.mult)
                    nc.vector.tensor_scalar(dst[:, kc, :], t1[:],
                                            g_t[:, kc:kc + 1],
                                            b_t[:, kc:kc + 1],
                                            OP.mult, OP.add)

            # ---- expert 0: SwiGLU --------------------------------------
            for ib in range(IB):
                for th in range(NTH):
                    a_r = blkp.tile([P, 4, 512], F32R, tag="blk")
                    fm(a_r, "sw_w1", 4, xr, th * 512, ACT.Silu, None,
                       w_col0=ib * 512)
                    b_r = blkp.tile([P, 4, 512], F32R, tag="blk")
                    fm(b_r, "sw_w3", 4, xr, th * 512, ACT.Copy, None,
                       w_col0=ib * 512)
                    nc.vector.tensor_tensor(b_r[:],
                                            a_r[:],
                                            b_r[:],
                                            OP.mult)
                    tm("sw_w2", ib * 4, 4, b_r, 0, th * 512,
                       init=(ib == 0))

            # ---- expert 3: GELU MLP ------------------------------------
            for ib in range(IB):
                for th in range(NTH):
                    a_r = blkp.tile([P, 4, 512], F32R, tag="blk")
                    fm(a_r, "ml_w1", 4, xr, th * 512, ACT.Gelu, ml_b1_t,
                       bias_col0=ib * 4, w_col0=ib * 512)
                    tm("ml_w2", ib * 4, 4, a_r, 3, th * 512, init=False,
                       bias_row=ml_b2_row if ib == 0 else None)

            # ---- expert 1: MathExpert ----------------------------------
            for sh in range(NTH):
                so = sh * 512
                eq = bigp.tile([P, KC, 512], F32R, tag="big")
                fm(eq, "me_eq_w", KC, xr, so, ACT.Copy, eq_b_t)
                v1 = bigp.tile([P, KC, 512], F32R, tag="big")
                fm(v1, "me_wv", KC, eq, 0, ACT.Copy, bv_t)
                sym = bigp.tile([P, KC, 512], F32R, tag="big")
                fm(sym, "me_wo", KC, v1, 0, ACT.Copy, bo_t)
                for cb in range(KC2 // 4):
                    c1 = blkp.tile([P, 4, 512], F32R, tag="blk")
                    fm(c1, "me_c1w", 4, sym, 0, ACT.Gelu, c1b_t,
                       bias_col0=cb * 4, w_col0=cb * 512)
                    tm("me_c2w", cb * 4, 4, c1, 1, so, init=False,
                       bias_row=c2b_row if cb == 0 else None)

            # ---- expert 2: CodeExpert ----------------------------------
            for sh in range(NTH):
                so = sh * 512
                syn = bigp.tile([P, KC, 512], F32R, tag="big")
                fm(syn, "ce_syn_w", KC, xr, so, ACT.Copy, syn_b_t)
                v = bigp.tile([P, KC, 512], F32R, tag="big")
                fm(v, "ce_wv", KC, syn, 0, ACT.Copy, cbv_t)
                at = bigp.tile([P, KC, 512], F32R, tag="big")
                fm(at, "ce_wo", KC, v, 0, ACT.Copy, cbo_t)
                nc.vector.tensor_tensor(syn[:],
                                        syn[:],
                                        at[:], OP.add)
                h2 = bigp.tile([P, KC, 512], F32R, tag="big")
                layer_norm(h2, syn, n1g_t, n1b_t)
                ffa = bigp.tile([P, KC, 512], F32R, tag="big")
                nc.vector.tensor_copy(ffa[:],
                                      h2[:])
                w2_all = wap(dt["ce_f2w"])
                for fb in range(KC2 // 4):
                    f1 = blkp.tile([P, 4, 512], F32R, tag="blk")
                    fm(f1, "ce_f1w", 4, h2, 0, ACT.Relu, f1b_t,
                       bias_col0=fb * 4, w_col0=fb * 512)
                    for half in range(HB):
                        wf = wpool.tile([P, 4, 512], F32, tag="w")
                        nc.sync.dma_start(
                            wf[:],
                            w2_all[:, fb * 4:fb * 4 + 4,
                                   half * 512:(half + 1) * 512])
                        wr = wrpool.tile([P, 4, 512], F32R, tag="wr")
                        cast(wr[:],
                             wf[:])
                        for ml in range(4):
                            mc = half * 4 + ml
                            ps = psp.tile([P, 512], F32, tag="mm")
                            for kc in range(4):
                                nc.tensor.matmul(
                                    ps[:], wr[:, kc, ml * P:(ml + 1) * P],
                                    f1[:, kc, :],
                                    start=(kc == 0), stop=(kc == 3))
                            if fb == 0:
                                nc.vector.scalar_tensor_tensor(
                                    ffa[:, mc, :], ps[:],
                                    f2b_t[:, mc:mc + 1], ffa[:, mc, :],
                                    OP.add, OP.add)
                            else:
                                nc.vector.tensor_tensor(
                                    ffa[:, mc, :], ps[:], ffa[:, mc, :],
                                    OP.add)
                h2b = bigp.tile([P, KC, 512], F32R, tag="big")
                layer_norm(h2b, ffa, n2g_t, n2b_t)
                tm("ce_gen_w", 0, KC, h2b, 2, so, init=False,
                   bias_row=gen_b_row)

            # ---- store -------------------------------------------------
            nc.sync.dma_start(
                out.ap().rearrange("(tt p) m -> p tt m", p=P), acc[:])

    nc.compile()
    return nc


_PROGRAM = None


def _get_program():
    global _PROGRAM
    if _PROGRAM is None:
        _PROGRAM = build_moe()
    return _PROGRAM


def run_cores(nc, in_maps, trace=False, trace_cores=None):
    if trace:
        _install_ntff_shim()
    return run_bass_kernel_spmd(nc, in_maps, core_ids=list(range(len(in_maps))),
                                trace=trace, trace_cores=trace_cores)


def make_in_maps(inputs):
    base = {n: np.ascontiguousarray(np.asarray(inputs[n], np.float32))
            for n in WEIGHT_NAMES}
    x = np.asarray(inputs["x"], np.float32).reshape(-1, H)
    in_maps = []
    for c in range(N_CORES):
        xt_c = np.ascontiguousarray(x[c * TOK:(c + 1) * TOK].T)
        in_maps.append({**base, "xt": xt_c})
    return in_maps


def kernel(**inputs):
    nc = _get_program()
    res = run_cores(nc, make_in_maps(inputs))
    outs = [res.results[c]["out"] for c in range(N_CORES)]
    x = np.asarray(inputs["x"])
    return np.concatenate(outs, 0).reshape(x.shape).astype(np.float32)


# ---- NTFF profiling shim (axon) — used by test.py only ----------------
def _install_ntff_shim():
    import contextlib
    import ctypes
    import sys
    import types

    if "antenv.axon_hooks" in sys.modules:
        return
    lib = ctypes.CDLL("/opt/axon/libaxon_pjrt.so")
    if not hasattr(lib, "axon_start_nrt_profile"):
        return
    lib.axon_start_nrt_profile.argtypes = [ctypes.POINTER(ctypes.c_int64),
                                           ctypes.c_size_t]
    lib.axon_start_nrt_profile.restype = ctypes.c_int64
    lib.axon_stop_nrt_profile.argtypes = [ctypes.c_char_p]
    lib.axon_stop_nrt_profile.restype = ctypes.c_int64

    @contextlib.contextmanager
    def _hook(output_dir, device_ids):
        import jax
        jax.devices()
        if device_ids:
            ids = (ctypes.c_int64 * len(device_ids))(*device_ids)
            rc = lib.axon_start_nrt_profile(ids, len(device_ids))
        else:
            rc = lib.axon_start_nrt_profile(None, 0)
        if rc != 0:
            raise RuntimeError(f"axon_start_nrt_profile rc={rc}")
        try:
            yield
        finally:
            n = lib.axon_stop_nrt_profile(str(output_dir).encode())
            print(f"profile: {n} file(s) written to {output_dir}",
                  file=sys.stderr)

    import antenv
    mod = types.ModuleType("antenv.axon_hooks")
    mod.get_axon_ntff_profile_hook = lambda: _hook
    mod.set_axon_ntff_profile_hook = lambda hk: None
    sys.modules["antenv.axon_hooks"] = mod
    antenv.axon_hooks = mod
